# revision 42
# baseline (speedup 1.0000x reference)
"""GNN message-passing attention kernel for Trainium2 (8 NeuronCores).

Problem: nn_Atten_1116691497442
  v   [E=500000, H=8, C=8, D=4] f32   edge values
  k   [E, H, S=32] f32                edge keys
  q   [N=20000, H, S] f32             node queries
  dst [E] int32                       destination node per edge
  out [N, H*C=64, D=4] f32            softmax(k.q[dst]) weighted scatter-sum

Math note: reference subtracts the per-node segment max before exp, which is
mathematically a no-op for the softmax ratio (logits are ~N(0, 0.35^2), so
exp never overflows).  We therefore compute, per edge, ex = exp(scale*k.q[dst])
and per node out = (sum ex*v) / max(sum ex, 1e-9) in ONE pass over edges.

Sharding: edges are sorted by dst on the host; nodes are partitioned across
the 8 cores (2500 nodes each) and every core receives exactly the edges that
target its nodes (graph partitioning; no halo needed because q-gather is local
after the sort, and no collectives at all).

Per-core device kernel, per 128-node window, per 128-edge tile:
  O[e,n]   = (dst_local[e] == n)            DVE  is_equal vs iota, bf16
  OT       = O^T                            PE   transpose -> PSUM, ACT copy
  Qg       = OT.T @ q_win = O @ q_win       PE   (bf16 gather of q rows)
  qg_sb    = bf16(Qg)                       ACT  PSUM -> SBUF copy
  prod     = k ⊙ qg_sb                      DVE  bf16 2x mode
  e8[e,h]  = sum_s prod[e,h,s]              DVE  grouped reduce, bf16 2x
  ex       = exp(scale*e8)                  ACT
  msg      = v ⊙ ex (bcast per head)        GPSIMD
  psum    += O.T @ msg ; den += O.T @ ex    PE   (scatter, accumulate f32)
window flush:  out = num * reciprocal(max(den,1e-9))  -> DMA out.

k/v/dst are streamed with partition-major DRAM layouts so each window is one
long contiguous DMA run per partition (descriptor-efficient); emission is
software-pipelined (gather of batch i+1 before scatter of batch i) so the
in-order PE stream never serializes the cross-engine chain.
"""

import contextlib
import math
import os

import numpy as np
import ml_dtypes

import concourse.bass as bass
import concourse.bacc as bacc
import concourse.tile as tile
from concourse import mybir
from concourse.bass_utils import run_bass_kernel_spmd
from concourse.masks import make_identity

F32 = mybir.dt.float32
BF16 = mybir.dt.bfloat16
I32 = mybir.dt.int32

P = 128          # partitions / edge-tile size / node-window size
HS = 256         # H*S per-edge key width
H = 8
S = 32
CD = 32          # C*D per-head value width
HCD = 256        # H*C*D per-edge value width

N_CORES = 8
N_NODES = 20000
N_EDGES = 500000
SCALE = 1.0 / 16.0   # 1/sqrt(H*S)

# batch of edge-tiles processed by one DVE/ACT/GPSIMD instruction
BATCH = 2
# where the big elementwise ops run (tunable)
MSG_ENGINE = os.environ.get("GNN_MSG_ENGINE", "gpsimd")   # gpsimd | vector
PROD_OUT_DT = os.environ.get("GNN_PROD_DT", "f32")        # f32 | bf16
V_DT = os.environ.get("GNN_V_DT", "bf16")                 # bf16 | f32
V_NP = ml_dtypes.bfloat16 if V_DT == "bf16" else np.float32
# fast path: k shipped bf16, gather-result copied to SBUF bf16 by ScalarE,
# logits reduced in bf16 -> the two big DVE ops hit the 2x perf mode
FAST2X = os.environ.get("GNN_FAST2X", "1") == "1"
K_NP = ml_dtypes.bfloat16 if FAST2X else np.float32


def build_layout(dst: np.ndarray, n_cores: int = N_CORES, n_nodes: int = N_NODES):
    """Sort edges by dst, partition nodes (and their edges) across cores,
    compute the shared (SPMD-identical) window/tile structure."""
    order = np.argsort(dst, kind="stable")
    dst_sorted = dst[order].astype(np.int64)

    nodes_per_core = (n_nodes + n_cores - 1) // n_cores          # 2500
    n_win = (nodes_per_core + P - 1) // P                        # 20
    win_nodes = [min(P, nodes_per_core - w * P) for w in range(n_win)]

    # edge count for every (core, window)
    counts = np.zeros((n_cores, n_win), dtype=np.int64)
    starts = np.zeros((n_cores, n_win), dtype=np.int64)
    for c in range(n_cores):
        base = c * nodes_per_core
        for w in range(n_win):
            lo = base + w * P
            hi = base + w * P + win_nodes[w]
            s = np.searchsorted(dst_sorted, lo, side="left")
            e = np.searchsorted(dst_sorted, hi, side="left")
            starts[c, w] = s
            counts[c, w] = e - s

    # tiles per window: shared across cores so the program is SPMD-identical
    T = [max(1, int(math.ceil(counts[:, w].max() / P))) for w in range(n_win)]
    Ttot = int(np.sum(T))
    return {
        "order": order,
        "dst_sorted": dst_sorted,
        "nodes_per_core": nodes_per_core,
        "n_win": n_win,
        "win_nodes": win_nodes,
        "counts": counts,
        "starts": starts,
        "T": T,
        "Ttot": Ttot,
    }


def build_nc(n_win, win_nodes, T, Ttot, batch=BATCH, loop_reps=1):
    """Build the per-core Bass program (identical on all cores).

    loop_reps > 1 wraps the whole computation in a hardware For_i loop that
    repeats it (same data, same output) — benchmark-only variant used to
    difference out host/dispatch overhead when measuring HW exec time.
    """
    nc = bacc.Bacc()
    # partition-major layouts: one whole-window DMA per stream gives each
    # partition a single long contiguous DRAM run (descriptor-efficient)
    k_d = nc.dram_tensor("k", [P, Ttot, HS], BF16 if FAST2X else F32,
                         kind="ExternalInput")
    v_d = nc.dram_tensor("v", [P, Ttot, HCD],
                         BF16 if V_DT == "bf16" else F32, kind="ExternalInput")
    dstl_d = nc.dram_tensor("dstl", [P, Ttot], F32, kind="ExternalInput")
    q_d = nc.dram_tensor("q", [n_win * P, HS], BF16, kind="ExternalInput")
    out_d = nc.dram_tensor("out", [n_win * P, HCD], F32, kind="ExternalOutput")

    prod_dt = F32 if PROD_OUT_DT == "f32" else BF16
    msg_eng_attr = MSG_ENGINE

    with tile.TileContext(nc) as tc:
        with (
            tc.tile_pool(name="const", bufs=1) as constp,
            tc.tile_pool(name="qwin", bufs=2) as qwinp,
            tc.tile_pool(name="kv", bufs=2) as kvp,
            tc.tile_pool(name="dstl", bufs=2) as dstlp,
            tc.tile_pool(name="oh", bufs=2 * batch + 2) as ohp,
            tc.tile_pool(name="otsb", bufs=3) as otsbp,
            tc.tile_pool(name="work", bufs=3) as workp,
            tc.tile_pool(name="flush", bufs=2) as flushp,
            tc.tile_pool(name="psq", bufs=2, space="PSUM") as psqp,
            tc.tile_pool(name="psot", bufs=2, space="PSUM") as psotp,
            tc.tile_pool(name="psout", bufs=2, space="PSUM") as psoutp,
            tc.tile_pool(name="psden", bufs=2, space="PSUM") as psdenp,
        ):
            # constants
            iota_i = constp.tile([P, P], I32, tag="iota_i")
            nc.gpsimd.iota(iota_i[:], pattern=[[1, P]], base=0, channel_multiplier=0)
            iota_bf = constp.tile([P, P], BF16, tag="iota_bf")
            nc.gpsimd.tensor_copy(iota_bf[:], iota_i[:])
            ident = constp.tile([P, P], BF16, tag="ident")
            make_identity(nc, ident[:])

            loop_ctx = (
                tc.For_i(0, loop_reps, 1) if loop_reps > 1
                else contextlib.nullcontext()
            )
            with loop_ctx:
              t_global = 0
              for w in range(n_win):
                Tw = T[w]
                t0w = t_global
                q_win = qwinp.tile([P, HS], BF16, tag="qwin")
                nc.sync.dma_start(out=q_win[:], in_=q_d[w * P:(w + 1) * P, :])

                # whole-window streams: one long contiguous run per partition
                kw = kvp.tile([P, Tw * HS], BF16 if FAST2X else F32, tag="kw")
                nc.sync.dma_start(
                    out=kw[:].rearrange("p (t s) -> p t s", t=Tw),
                    in_=k_d[:, t0w:t0w + Tw, :],
                )
                vw = kvp.tile([P, Tw * HCD],
                              BF16 if V_DT == "bf16" else F32, tag="vw")
                nc.sync.dma_start(
                    out=vw[:].rearrange("p (t s) -> p t s", t=Tw),
                    in_=v_d[:, t0w:t0w + Tw, :],
                )
                dstl_raw = dstlp.tile([P, Tw], F32, tag="dstl_raw")
                nc.sync.dma_start(
                    out=dstl_raw[:], in_=dstl_d[:, t0w:t0w + Tw]
                )
                # engine-local copy so the TensorScalarPtr one-hot build
                # never needs its own DMA semaphore wait
                dstl_w = dstlp.tile([P, Tw], F32, tag="dstl")
                nc.vector.tensor_copy(out=dstl_w[:], in_=dstl_raw[:])

                ps_out = psoutp.tile([P, HCD], F32, tag="psout")
                ps_den = psdenp.tile([P, H], F32, tag="psden")

                batches = []
                j0 = 0
                while j0 < Tw:
                    batches.append((j0, min(batch, Tw - j0)))
                    j0 += batches[-1][1]
                nb = len(batches)
                state = [None] * nb

                def emit_gather(i):
                    j0, b = batches[i]
                    qg_ps = psqp.tile([P, b * HS], F32, space="PSUM", tag="qg")
                    O_tiles = []
                    for j in range(b):
                        O_j = ohp.tile([P, P], BF16, tag="O")
                        nc.vector.tensor_scalar(
                            out=O_j[:],
                            in0=iota_bf[:],
                            scalar1=dstl_w[:, j0 + j:j0 + j + 1],
                            scalar2=None,
                            op0=mybir.AluOpType.is_equal,
                        )
                        O_tiles.append(O_j)
                        ot_ps = psotp.tile([P, P], BF16, space="PSUM", tag="ot")
                        nc.tensor.transpose(
                            out=ot_ps[:], in_=O_j[:], identity=ident[:]
                        )
                        ot_sb = otsbp.tile([P, P], BF16, tag="otsb")
                        nc.scalar.copy(out=ot_sb[:], in_=ot_ps[:])
                        nc.tensor.matmul(
                            out=qg_ps[:, j * HS:(j + 1) * HS],
                            lhsT=ot_sb[:],
                            rhs=q_win[:],
                            start=True,
                            stop=True,
                        )
                    state[i] = {"qg": qg_ps, "O": O_tiles}

                def emit_math(i):
                    j0, b = batches[i]
                    st = state[i]
                    if FAST2X:
                        # ScalarE copies the PSUM gather result to SBUF bf16
                        # so prod and reduce both run in the DVE 2x mode
                        qg_sb = workp.tile([P, b * HS], BF16, tag="qgsb")
                        nc.scalar.copy(out=qg_sb[:], in_=st["qg"][:])
                        prod_in1 = qg_sb[:]
                        pdt = BF16
                    else:
                        prod_in1 = st["qg"][:]
                        pdt = prod_dt
                    prod = workp.tile([P, b * HS], pdt, tag="prod")
                    nc.vector.tensor_tensor(
                        out=prod[:], in0=kw[:, j0 * HS:(j0 + b) * HS],
                        in1=prod_in1,
                        op=mybir.AluOpType.mult,
                    )
                    e8 = workp.tile([P, b * H], BF16 if FAST2X else F32,
                                    tag="e8")
                    with (nc.allow_low_precision("e8 |.|<=32 feeds exp; "
                                                 "bf16 rounding ~0.4%")
                          if FAST2X else contextlib.nullcontext()):
                        nc.vector.tensor_reduce(
                            out=e8[:].rearrange("p (b h) -> p b h", h=H),
                            in_=prod[:].rearrange("p (b h s) -> p b h s",
                                                  h=H, s=S),
                            axis=mybir.AxisListType.X,
                            op=mybir.AluOpType.add,
                        )
                    ex = workp.tile([P, b * H], BF16, tag="ex")
                    nc.scalar.activation(
                        out=ex[:], in_=e8[:],
                        func=mybir.ActivationFunctionType.Exp,
                        scale=SCALE,
                    )
                    msg = workp.tile([P, b * HCD], BF16, tag="msg")
                    ex_b = (
                        ex[:]
                        .rearrange("p (b h) -> p b h", h=H)[:, :, :, None]
                        .to_broadcast([P, b, H, CD])
                    )
                    msg_eng = (nc.gpsimd if msg_eng_attr == "gpsimd"
                               else nc.vector)
                    msg_eng.tensor_tensor(
                        out=msg[:].rearrange("p (b h s) -> p b h s", h=H, s=CD),
                        in0=vw[:, j0 * HCD:(j0 + b) * HCD].rearrange(
                            "p (b h s) -> p b h s", h=H, s=CD
                        ),
                        in1=ex_b,
                        op=mybir.AluOpType.mult,
                    )
                    st["msg"] = msg
                    st["ex"] = ex

                def emit_scatter(i):
                    j0, b = batches[i]
                    st = state[i]
                    for j in range(b):
                        t = j0 + j
                        nc.tensor.matmul(
                            out=ps_out[:],
                            lhsT=st["O"][j][:],
                            rhs=st["msg"][:, j * HCD:(j + 1) * HCD],
                            start=(t == 0),
                            stop=(t == Tw - 1),
                        )
                        nc.tensor.matmul(
                            out=ps_den[:],
                            lhsT=st["O"][j][:],
                            rhs=st["ex"][:, j * H:(j + 1) * H],
                            start=(t == 0),
                            stop=(t == Tw - 1),
                        )
                    state[i] = None

                # software pipeline: gather of batch i+1 is emitted before
                # the scatter of batch i so the in-order PE stream never
                # stalls the next batch behind the cross-engine chain
                emit_gather(0)
                for i in range(nb):
                    emit_math(i)
                    if i + 1 < nb:
                        emit_gather(i + 1)
                    emit_scatter(i)
                t_global += Tw

                # ---- window flush ----
                den_r = flushp.tile([P, H], F32, tag="denr")
                nc.vector.tensor_scalar(
                    out=den_r[:], in0=ps_den[:],
                    scalar1=1e-9, scalar2=None,
                    op0=mybir.AluOpType.max,
                )
                nc.vector.reciprocal(den_r[:], den_r[:])
                out_sb = flushp.tile([P, HCD], F32, tag="outsb")
                nc.vector.tensor_tensor(
                    out=out_sb[:].rearrange("p (h s) -> p h s", h=H),
                    in0=ps_out[:].rearrange("p (h s) -> p h s", h=H),
                    in1=den_r[:][:, :, None].to_broadcast([P, H, CD]),
                    op=mybir.AluOpType.mult,
                )
                rows = win_nodes[w]
                nc.sync.dma_start(
                    out=out_d[w * P:w * P + rows, :], in_=out_sb[:rows, :]
                )

    nc.finalize()
    return nc


def prepare_inputs(v, k, q, dst, layout, n_cores=N_CORES):
    """Host-side shard: permute edges into sorted, window-padded per-core
    arrays.  Padded edge slots get dst_local = -1 (one-hot row of zeros) so
    they contribute nothing on device."""
    order = layout["order"]
    dst_sorted = layout["dst_sorted"]
    npc = layout["nodes_per_core"]
    n_win = layout["n_win"]
    T = layout["T"]
    Ttot = layout["Ttot"]
    counts = layout["counts"]
    starts = layout["starts"]

    k_flat = np.ascontiguousarray(k.reshape(k.shape[0], -1))
    v_flat = np.ascontiguousarray(v.reshape(v.shape[0], -1))
    ks = k_flat[order].astype(K_NP)
    vs = v_flat[order].astype(V_NP)
    q_flat = np.ascontiguousarray(q.reshape(q.shape[0], -1)).astype(np.float32)

    # tile offsets of each window in the padded layout
    w_off = np.concatenate([[0], np.cumsum(T)]).astype(np.int64)

    in_maps = []
    for c in range(n_cores):
        k_pad = np.zeros((Ttot * P, HS), dtype=K_NP)
        v_pad = np.zeros((Ttot * P, HCD), dtype=V_NP)
        dstl = np.full((Ttot, P), -1.0, dtype=np.float32)
        q_c = np.zeros((n_win * P, HS), dtype=ml_dtypes.bfloat16)
        q_c[:npc] = q_flat[c * npc:(c + 1) * npc]
        for w in range(n_win):
            s = starts[c, w]
            n_e = counts[c, w]
            off = w_off[w] * P
            k_pad[off:off + n_e] = ks[s:s + n_e]
            v_pad[off:off + n_e] = vs[s:s + n_e]
            loc = (dst_sorted[s:s + n_e] - (c * npc + w * P)).astype(np.float32)
            dstl.reshape(-1)[off:off + n_e] = loc
        # partition-major device layouts (edge-within-tile on the partition
        # axis) so whole-window DMAs are one long run per partition
        k_pm = np.ascontiguousarray(
            k_pad.reshape(Ttot, P, HS).transpose(1, 0, 2))
        v_pm = np.ascontiguousarray(
            v_pad.reshape(Ttot, P, HCD).transpose(1, 0, 2))
        dstl_pm = np.ascontiguousarray(dstl.T)
        in_maps.append({"k": k_pm, "v": v_pm, "dstl": dstl_pm, "q": q_c})
    return in_maps


_CACHE = {}


def run_full(v, k, q, dst, **run_kwargs):
    v = np.asarray(v, dtype=np.float32)
    k = np.asarray(k, dtype=np.float32)
    q = np.asarray(q, dtype=np.float32)
    dst = np.asarray(dst, dtype=np.int32)

    layout = build_layout(dst)
    in_maps = prepare_inputs(v, k, q, dst, layout)

    key = tuple(layout["T"]), layout["n_win"]
    if key not in _CACHE:
        _CACHE[key] = build_nc(
            layout["n_win"], layout["win_nodes"], layout["T"], layout["Ttot"]
        )
    nc = _CACHE[key]

    res = run_bass_kernel_spmd(nc, in_maps, core_ids=list(range(N_CORES)),
                               **run_kwargs)
    npc = layout["nodes_per_core"]
    out = np.empty((N_NODES, HCD), dtype=np.float32)
    for c in range(N_CORES):
        out[c * npc:(c + 1) * npc] = res.results[c]["out"][:npc]
    return out.reshape(N_NODES, H * 8, 4), res


def kernel(v, k, q, dst):
    out, _ = run_full(v, k, q, dst)
    return out
